# revision 13
# baseline (speedup 1.0000x reference)
"""TransformerXL relative attention on 8 TRN2 NeuronCores — v2.

Sharding: DP over batch (4 groups of 2 cores) x TP over heads (2-way, 8
heads each).  Core c handles batch b=c//2, head group g=c%2.  Each core
computes a partial output-projection (512,1024); host sums the pair.

v2 redesign vs v1 (254us -> 130us in the TimelineSim cost model):
- Only the valid causal band (hull) of the scores is computed, written,
  and read back: position matmul window per query tile qt is
  m in [384-128*qt, 1024), shifted read window j in [0, 128*(qt+5));
  pad columns [1024,1536) of the DRAM shift buffer hold BIG once, so the
  causal mask falls out of the shifted read for free.
- The shifted position rows are read back with a SWDGE (gpsimd) DMA using
  accum_op=add, accumulating directly onto the content scores in SBUF --
  no vector-engine add pass at all.
- PSUM evictions are split across the vector engine (position scores) and
  the scalar engine as Copy activations (content scores; Copy and Exp
  share an act table set so there are no table reloads), keeping both
  engines just under the PE's per-head cadence and keeping the Act queue
  free of copies between the transpose-exp instructions.
- Transposes/exp/A@V restricted to the hull; the A@V accumulation uses a
  single full-width start at jt=0 (PSUM start clears the whole bank) and
  narrows the free range for the later key blocks.
- Inputs split into per-matrix (and per-k8-slice) DMAs ordered R, Q, K, V
  so projections chase the loads; V-projection groups are interleaved
  into the first two head slots as PE filler while their DMA round trips
  are in flight; 2-deep software pipeline across heads hides the rest.
- Output projection interleaves with the last head's chunked softmax
  normalization, alternating two PSUM rings and two eviction engines.
"""

import numpy as np
import ml_dtypes

import concourse.bass as bass
import concourse.mybir as mybir
import concourse.tile as tile
from concourse import bacc
from concourse.bass_utils import run_bass_kernel_spmd
from concourse.masks import make_identity

CUR, FULL, BS, DIM, H, D = 512, 1024, 4, 1024, 16, 64
NHC = 8                 # heads per core
HDC = NHC * D           # 512 head-dims per core
SCALE = 1.0 / D ** 0.5  # 0.125
BIG = -30000.0
PADW = 1536             # padded row width for the shift round trip
RSTR = PADW - 1         # shifted read row stride
BF = mybir.dt.bfloat16
F32 = mybir.dt.float32
Exp = mybir.ActivationFunctionType.Exp
Copy = mybir.ActivationFunctionType.Copy
AluAdd = mybir.AluOpType.add
USE_ACCUM_DMA = True

_CACHED = {}


def build_program():
    nc = bacc.Bacc(None, target_bir_lowering=False, debug=False)
    ins = {}
    for name, shape in [
        ("xfull", [DIM, FULL]), ("xcur", [DIM, CUR]), ("xpos", [DIM, FULL]),
        ("wq", [DIM, HDC]), ("wk", [DIM, HDC]), ("wv", [DIM, HDC]),
        ("wpos", [DIM, HDC]), ("wproj", [HDC, DIM]), ("uvall", [1, 2 * HDC]),
    ]:
        ins[name] = nc.declare_dram_parameter(name, shape, BF, isOutput=False)
    outp = nc.declare_dram_parameter("outp", [CUR, DIM], F32, isOutput=True)

    with tile.TileContext(nc) as tc:
        const = tc.alloc_tile_pool(name="const", bufs=1)
        psA = tc.alloc_tile_pool(name="psA", bufs=3, space="PSUM")
        psB = tc.alloc_tile_pool(name="psB", bufs=2, space="PSUM")
        psT = tc.alloc_tile_pool(name="psT", bufs=2, space="PSUM")
        psO = tc.alloc_tile_pool(name="psO", bufs=1, space="PSUM")
        slabp = tc.alloc_tile_pool(name="slabp", bufs=2)
        sp = tc.alloc_tile_pool(name="sp", bufs=2)
        atp = tc.alloc_tile_pool(name="atp", bufs=2)
        work = tc.alloc_tile_pool(name="work", bufs=2)
        dram = tc.alloc_tile_pool(name="dram", bufs=2, space="DRAM")

        # ---- resident SBUF tensors ----
        xfull_sb = const.tile([128, 8, FULL], BF)
        xcur_sb = const.tile([128, 8, CUR], BF)
        xpos_sb = const.tile([128, 8, FULL], BF)
        w_sb = {}
        for wname in ("wq", "wk", "wv", "wpos"):
            w_sb[wname] = const.tile([128, 8, HDC], BF, name=wname + "_sb")
        wproj_sb = const.tile([128, 4, DIM], BF)
        uvall_sb = const.tile([1, 2 * HDC], BF)
        ident = const.tile([128, 128], BF)
        ones_row = const.tile([1, CUR], BF)
        big_sb = const.tile([128, 512], BF)
        kt_sb = const.tile([128, 4, FULL], BF)    # K^T (dc, j)
        rt_sb = const.tile([128, 4, FULL], BF)    # R^T (dc, m)
        v_sb = const.tile([128, 8, NHC, D + 1], BF)
        qt_sb = const.tile([128, 4, 2, CUR], BF)  # Q^T with +u / +v
        o_sb = const.tile([128, 4, CUR], BF)      # O^T normalized (e, g, i)

        def rearr(t):
            return t[:].rearrange("(a p) n -> p a n", p=128)

        # split loads in first-use order; wpos/xpos/wk/xfull arrive in
        # per-k8 slices so the contraction loop can chase the DMAs
        for k8 in range(8):
            nc.sync.dma_start(out=w_sb["wpos"][:, k8:k8 + 1, :],
                              in_=rearr(ins["wpos"])[:, k8:k8 + 1, :])
            nc.sync.dma_start(out=xpos_sb[:, k8:k8 + 1, :],
                              in_=rearr(ins["xpos"])[:, k8:k8 + 1, :])
        nc.sync.dma_start(out=uvall_sb[:], in_=ins["uvall"][:])
        nc.sync.dma_start(out=w_sb["wq"][:], in_=rearr(ins["wq"]))
        nc.sync.dma_start(out=xcur_sb[:], in_=rearr(ins["xcur"]))
        for k8 in range(8):
            nc.sync.dma_start(out=w_sb["wk"][:, k8:k8 + 1, :],
                              in_=rearr(ins["wk"])[:, k8:k8 + 1, :])
            nc.sync.dma_start(out=xfull_sb[:, k8:k8 + 1, :],
                              in_=rearr(ins["xfull"])[:, k8:k8 + 1, :])
        nc.sync.dma_start(out=w_sb["wv"][:], in_=rearr(ins["wv"]))
        nc.sync.dma_start(out=wproj_sb[:], in_=rearr(ins["wproj"]))
        make_identity(nc, ident[:])
        nc.vector.memset(ones_row[:], 1.0)
        nc.vector.memset(big_sb[:], BIG)
        nc.gpsimd.memset(v_sb[:, :, :, D:D + 1], 1.0)

        # shift buffers: one per head slot (double-buffered across heads);
        # pad columns [1024,1536) hold BIG once (causal mask for free).
        # DRAM tiles are never ring-reused by the pool, so allocate the 2
        # buffers once and index them manually.
        pdram = []
        for rep in range(2):
            t = dram.tile([CUR * PADW], BF, tag=f"pd_{rep}",
                          name=f"pd_{rep}")
            pdram.append(t)
            for rq in range(4):
                nc.sync.dma_start(
                    out=bass.AP(tensor=t.tensor,
                                offset=rq * 128 * PADW + FULL,
                                ap=[[PADW, 128], [1, 512]]),
                    in_=big_sb[:])

        # ---- projections ----
        # R^T then Q (position side first), K (content), V last.
        def stage_r_start():
            """First 4 R^T groups k8-major across 4 PSUM tiles, so the PE
            chases the per-k8 input DMA slices instead of stalling on the
            first group's full contraction."""
            tiles = [psA.tile([128, 512], F32, tag="pj", name=f"p_r0_{g}")
                     for g in range(2)]
            tiles += [psB.tile([128, 512], F32, tag="pv", name=f"p_r1_{g}")
                      for g in range(2)]
            for k8 in range(8):
                for g, pk in enumerate(tiles):
                    t, nh = g // 2, g % 2
                    nc.tensor.matmul(
                        pk[:],
                        w_sb["wpos"][:, k8, t * 128:(t + 1) * 128],
                        xpos_sb[:, k8, nh * 512:(nh + 1) * 512],
                        start=(k8 == 0), stop=(k8 == 7))
            for g, pk in enumerate(tiles):
                t, nh = g // 2, g % 2
                nc.vector.tensor_copy(
                    rt_sb[:, t, nh * 512:(nh + 1) * 512], pk[:])

        def stage_kr(w, x_sb, dst, t0=0):
            for t in range(t0, 4):
                for nh in range(2):
                    pk = psA.tile([128, 512], F32, tag="pj",
                                  name=f"p_{w}_{t}_{nh}")
                    for k8 in range(8):
                        nc.tensor.matmul(
                            pk[:],
                            w_sb[w][:, k8, t * 128:(t + 1) * 128],
                            x_sb[:, k8, nh * 512:(nh + 1) * 512],
                            start=(k8 == 0), stop=(k8 == 7))
                    nc.vector.tensor_copy(
                        dst[:, t, nh * 512:(nh + 1) * 512], pk[:])

        stage_r_start()
        stage_kr("wpos", xpos_sb, rt_sb, t0=2)
        for t in range(4):
            pq = psA.tile([128, 512], F32, tag="pj", name=f"p_q_{t}")
            for k8 in range(8):
                nc.tensor.matmul(
                    pq[:], w_sb["wq"][:, k8, t * 128:(t + 1) * 128],
                    xcur_sb[:, k8, :],
                    start=(k8 == 0), stop=False)
            nc.tensor.matmul(pq[:], uvall_sb[0:1, t * 128:(t + 1) * 128],
                             ones_row[:], start=False, stop=False)
            nc.vector.tensor_copy(qt_sb[:, t, 0, :], pq[:])
            nc.tensor.matmul(pq[:],
                             uvall_sb[0:1, HDC + t * 128:HDC + (t + 1) * 128],
                             ones_row[:], start=False, stop=True)
            nc.vector.tensor_copy(qt_sb[:, t, 1, :], pq[:])
        stage_kr("wk", xfull_sb, kt_sb)

        def v_group(jt):
            pv = psB.tile([128, 512], F32, tag="pv", name=f"p_v_{jt}")
            for k8 in range(8):
                nc.tensor.matmul(
                    pv[:], xfull_sb[:, k8, jt * 128:(jt + 1) * 128],
                    w_sb["wv"][:, k8, :],
                    start=(k8 == 0), stop=(k8 == 7))
            nc.vector.tensor_copy(
                v_sb[:, jt, :, 0:D],
                pv[:].rearrange("p (h d) -> p h d", h=NHC))

        # ---- attention stages (per head) ----
        s_tiles = {}    # (h, qt) -> content+shifted-pos scores [128, jw]
        at_tiles = {}   # h -> [128, 8, CUR] A^T blocks

        def evict(engine, out, in_):
            if engine == "v":
                nc.vector.tensor_copy(out, in_)
            elif engine == "a":
                nc.scalar.activation(out, in_, Copy)
            else:
                nc.gpsimd.tensor_copy(out, in_)

        # PSUM-eviction engine assignment (GPSIMD has no PSUM access, so
        # only DVE "v" and Act "a"), balancing both per head
        SLAB_ENG = {(0, 0): "v", (1, 0): "v", (2, 0): "v", (3, 0): "v",
                    (0, 1): "v", (1, 1): "v", (2, 1): "v", (3, 1): "v"}
        CONT_ENG = {(0, 0): "a", (1, 0): "a", (2, 0): "a", (3, 0): "a",
                    (0, 1): "a", (1, 1): "a", (2, 1): "v", (3, 1): "v"}

        slab_tiles = {}

        def a_pos_qt(h, qt):
            """Position scores for one query tile: matmul, evict, write."""
            p0 = (h % 2) * 64
            th = h // 2
            if qt == 0:
                slab_tiles[h] = slabp.tile([128, 4, FULL], BF, tag="slab",
                                           name=f"slab_{h}")
            slab = slab_tiles[h]
            m_min = 384 - 128 * qt
            c = m_min
            ci = 0
            while c < 1024:
                ce = min(c + 512, 1024)
                pp = psA.tile([128, ce - c], F32, tag="pj",
                              name=f"pp_{h}_{qt}_{ci}")
                nc.tensor.matmul(
                    pp[:],
                    qt_sb[p0:p0 + 64, th, 1, qt * 128:(qt + 1) * 128],
                    rt_sb[p0:p0 + 64, th, c:ce],
                    start=True, stop=True)
                evict(SLAB_ENG[(qt, ci)], slab[:, qt, c:ce], pp[:])
                c = ce
                ci += 1
            pd = pdram[h % 2]
            nc.sync.dma_start(
                out=bass.AP(tensor=pd.tensor,
                            offset=qt * 128 * PADW + m_min,
                            ap=[[PADW, 128], [1, 1024 - m_min]]),
                in_=slab[:, qt, m_min:1024])

        def a_content_qt(h, qt):
            """Content scores for one tile + shifted-pos accumulate DMA."""
            p0 = (h % 2) * 64
            th = h // 2
            pd = pdram[h % 2]
            if qt == 0:
                s_tiles[h] = sp.tile([128, 4, FULL], BF, tag="s",
                                     name=f"s_{h}")
            s_all = s_tiles[h]
            jw = 640 + 128 * qt
            c = 0
            ci = 0
            while c < jw:
                ce = min(c + 512, jw)
                pc = psB.tile([128, ce - c], F32, tag="pv",
                              name=f"pc_{h}_{qt}_{ci}")
                nc.tensor.matmul(
                    pc[:],
                    qt_sb[p0:p0 + 64, th, 0, qt * 128:(qt + 1) * 128],
                    kt_sb[p0:p0 + 64, th, c:ce],
                    start=True, stop=True)
                evict(CONT_ENG[(qt, ci)], s_all[:, qt, c:ce], pc[:])
                c = ce
                ci += 1
            # shifted position rows accumulate onto the content scores
            nc.gpsimd.dma_start(
                out=s_all[:, qt, 0:jw],
                in_=bass.AP(tensor=pd.tensor,
                            offset=qt * 128 * PADW + 511 - 128 * qt,
                            ap=[[RSTR, 128], [1, jw]]),
                accum_op=AluAdd)

        def e_qt(h, qt, tpool=None):
            """Transpose hull blocks of one tile, exponentiate into A^T."""
            if qt == 0:
                at_all = atp.tile([128, 8, CUR], BF, tag="at",
                                  name=f"at_{h}")
                at_tiles[h] = at_all
            at_all = at_tiles[h]
            s_all = s_tiles[h]
            nj8 = qt + 5
            st = (tpool or psT).tile([128, 8, 128], BF,
                                     tag="pj" if tpool else "pt",
                                     name=f"st_{h}_{qt}")
            for j8 in range(nj8):
                nc.tensor.transpose(st[:, j8, :],
                                    s_all[:, qt, j8 * 128:(j8 + 1) * 128],
                                    ident[:])
            nc.scalar.activation(
                at_all[:, 0:nj8, qt * 128:(qt + 1) * 128],
                st[:, 0:nj8, :], Exp, scale=SCALE)

        def f_av(h, jts, pool=None):
            """Part of A^T @ V accumulation (ones column -> denominator)."""
            if jts[0] == 0:
                ov_tiles[h] = (pool or psO).tile(
                    [D + 1, CUR], F32, tag="pv" if pool else "po",
                    name=f"ov_{h}")
            ov = ov_tiles[h]
            at_all = at_tiles[h]
            for jt in jts:
                # hull-restricted: block jt only attends to queries
                # i >= (jt-4)*128; a single full-width start at jt=0 keeps
                # PSUM accumulation semantics safe (start clears the bank)
                c0 = max(0, (jt - 4) * 128)
                nc.tensor.matmul(ov[:, c0:], v_sb[:, jt, h, :],
                                 at_all[:, jt, c0:],
                                 start=(jt == 0), stop=(jt == 7),
                                 skip_group_check=True)

        def f_norm(h, chunks=((0, CUR),), done=True):
            """Normalize by the softmax denominator into O^T."""
            p0 = (h % 2) * 64
            th = h // 2
            ov = ov_tiles[h]
            if done:
                ov_tiles.pop(h)
                at_tiles.pop(h, None)
            for c0, c1 in chunks:
                rden = work.tile([1, CUR], F32, tag="rden", bufs=4,
                                 name=f"rden_{h}_{c0}")
                nc.vector.reciprocal(rden[0:1, 0:c1 - c0], ov[D:D + 1, c0:c1])
                rdb = work.tile([64, CUR], F32, tag="rdb", bufs=4,
                                name=f"rdb_{h}_{c0}")
                nc.gpsimd.partition_broadcast(rdb[0:64, 0:c1 - c0],
                                              rden[0:1, 0:c1 - c0])
                nc.vector.tensor_mul(o_sb[p0:p0 + 64, th, c0:c1],
                                     ov[0:D, c0:c1], rdb[0:64, 0:c1 - c0])

        ov_tiles = {}

        def head_slot(ha, he, vjts=()):
            """One pipeline slot: scores for head ha interleaved with
            transpose/exp/AV for head he (and V-projection filler groups),
            paced at qt granularity."""
            vit = iter(vjts)
            for qt in range(4):
                if he is not None:
                    e_qt(he, qt)
                v = next(vit, None)
                if v is not None:
                    v_group(v)
                if ha is not None:
                    a_pos_qt(ha, qt)
            for qt in range(4):
                v = next(vit, None)
                if v is not None:
                    v_group(v)
                if ha is not None:
                    a_content_qt(ha, qt)
                if he is not None and qt % 2 == 1:
                    f_av(he, ((qt - 1) * 2, (qt - 1) * 2 + 1,
                              (qt - 1) * 2 + 2, (qt - 1) * 2 + 3))
            if he is not None:
                f_norm(he)

        # ---- output projection, interleaved with head 7's norm chunks ----
        proj_tiles = {}

        def proj_start(qt, pools=None, nhs=(0, 1)):
            """Allocate accumulators for output tile qt and run the et<3
            matmuls (heads 0-5, ready long before the tail)."""
            for nh in nhs:
                pool, tg = (pools or ((psB, "pv"), (psO, "po")))[nh]
                pr = pool.tile([128, 512], F32, tag=tg, name=f"pr_{qt}_{nh}")
                proj_tiles[(qt, nh)] = pr
                for et in (0, 1, 2):
                    nc.tensor.matmul(
                        pr[:], o_sb[:, et, qt * 128:(qt + 1) * 128],
                        wproj_sb[:, et, nh * 512:(nh + 1) * 512],
                        start=(et == 0), stop=False)

        def proj_finish(qt):
            ot = work.tile([128, FULL], F32, tag="ot", bufs=4, name=f"ot_{qt}")
            for nh in range(2):
                if (qt, nh) not in proj_tiles:
                    proj_start(qt, nhs=(nh,))
                pr = proj_tiles.pop((qt, nh))
                nc.tensor.matmul(
                    pr[:], o_sb[:, 3, qt * 128:(qt + 1) * 128],
                    wproj_sb[:, 3, nh * 512:(nh + 1) * 512],
                    start=False, stop=True)
                evict("a" if nh == 0 else "v",
                      ot[:, nh * 512:(nh + 1) * 512], pr[:])
                nc.sync.dma_start(
                    out=outp[qt * 128:(qt + 1) * 128,
                             nh * 512:(nh + 1) * 512],
                    in_=ot[:, nh * 512:(nh + 1) * 512])

        def proj_qt(qt):
            proj_start(qt)
            proj_finish(qt)

        # software pipeline: heads 2 deep over the DMA round trip
        head_slot(0, None, vjts=(0, 1))
        head_slot(1, None, vjts=(2, 3, 4, 5, 6, 7))
        for h in range(2, NHC):
            head_slot(h, h - 2)
        # tail: interleave the last two heads; head 7's accumulator lives
        # in the (now idle) content ring so both can be in flight
        h6, h7 = NHC - 2, NHC - 1
        for qt in range(4):
            e_qt(h6, qt)
        e_qt(h7, 0)
        e_qt(h7, 1)
        f_av(h6, (0, 1, 2, 3))
        e_qt(h7, 2)
        f_av(h6, (4, 5, 6, 7))
        e_qt(h7, 3)
        proj_start(0, pools=((psA, "pj"), (psA, "pj")))
        f_norm(h6)
        f_av(h7, (0, 1, 2, 3), pool=psB)
        f_av(h7, (4, 5, 6, 7), pool=psB)


        for qt in range(4):
            f_norm(h7, chunks=((qt * 128, (qt + 1) * 128),), done=(qt == 3))
            if qt == 0:
                proj_finish(0)
            else:
                proj_qt(qt)

        for p in (dram, work, atp, sp, slabp, psO, psT, psB, psA, const):
            p.release()
    nc.compile()
    return nc


def kernel(inputs, pos_embedding, full_input, u, v, W_kv, b_kv, W_q, b_q,
           W_pos, b_pos, W_proj, b_proj, mask):
    bf = ml_dtypes.bfloat16
    inputs = np.asarray(inputs)
    full_input = np.asarray(full_input)
    pos = np.asarray(pos_embedding)[:, 0, :]

    if "nc" not in _CACHED:
        _CACHED["nc"] = build_program()
    nc = _CACHED["nc"]

    in_maps = []
    for c in range(8):
        b, g = c // 2, c % 2
        sl = slice(g * HDC, (g + 1) * HDC)
        uvec = (np.asarray(u).reshape(-1) + np.asarray(b_q))[sl]
        vvec = (np.asarray(v).reshape(-1) + np.asarray(b_q))[sl]
        in_maps.append({
            "xfull": np.ascontiguousarray(full_input[:, b, :].T).astype(bf),
            "xcur": np.ascontiguousarray(inputs[:, b, :].T).astype(bf),
            "xpos": np.ascontiguousarray(pos.T).astype(bf),
            "wq": np.ascontiguousarray(W_q[:, sl]).astype(bf),
            "wk": np.ascontiguousarray(W_kv[:, g * HDC:(g + 1) * HDC]).astype(bf),
            "wv": np.ascontiguousarray(
                W_kv[:, H * D + g * HDC:H * D + (g + 1) * HDC]).astype(bf),
            "wpos": np.ascontiguousarray(W_pos[:, sl]).astype(bf),
            "wproj": np.ascontiguousarray(W_proj[sl, :]).astype(bf),
            "uvall": np.concatenate([uvec, vvec - uvec])[None, :].astype(bf),
        })

    _CACHED["maps"] = in_maps
    res = run_bass_kernel_spmd(nc, in_maps, list(range(8)))
    out = np.empty((CUR, BS, DIM), np.float32)
    for b in range(BS):
        out[:, b, :] = res.results[2 * b]["outp"] + res.results[2 * b + 1]["outp"]
    return out
